# revision 36
# baseline (speedup 1.0000x reference)
"""Trainium2 Bass kernel for GQA fractal attention (B=2, L=2048, D=1024,
8 heads, 2 query groups, fractal per-key-group scale, masked softmax,
output projection, residual + LayerNorm).

Sharding: 8 cores = batch (2) x query-chunk (4 x 512 rows). Each core
computes the FULL K^T for its batch locally (cheaper than an AllGather
round-trip on this problem size), projects V for its own 512 keys and
AllGathers the full V within its batch quad (V is only needed late, so the
collective latency hides behind the scores/softmax pipeline), then computes
scores/attention for its 512 query rows and output projection + residual +
LayerNorm for those rows.

All matmuls run as fp8e4m3 DoubleRow (two stacked contraction rows per
cycle-pair -- 4x bf16 throughput on the PE array). Scale bookkeeping:
  - weights are uploaded x16 so fp8 stays out of the subnormal range;
  - exp applies a per-key-group bias e^{b_G} (1/4, 1/64) to keep
    unnormalized probabilities in fp8 range;
  - masking folds in host-side: masked rows of x are zeroed for the K/V
    projections (masked scores become exactly 0, masked V rows 0) and the
    softmax denominator matmul uses a masked-ones stationary vector;
  - the remaining scale factors collapse into one global factor R=1024 on
    the pre-LayerNorm activations, which LayerNorm removes exactly.
Softmax normalization is applied to the PV output (per-q reciprocal
broadcast by a rank-1 matmul), so P goes straight from the Exp activation
into the PV matmuls. A trickle of tiny warm-up matmuls keeps the PE p-state
at full clock through the DMA-paced projection phase.
"""

import math
import sys

if "/opt/trn_rl_repo" not in sys.path:
    sys.path.insert(0, "/opt/trn_rl_repo")

import ml_dtypes
import numpy as np

import concourse.bass as bass
import concourse.mybir as mybir
import concourse.tile as tile
from concourse.vector_clock import ScopedClock

# ---------------------------------------------------------------- constants
P = 128
L = 2048
D = 1024
NQ = 512          # query rows / keys per core
HEADS = 8
QG = 2            # query groups
GD = 512          # per-group feature dim (4 heads x 128)
SCALE = (D // HEADS) ** -0.5
LN_EPS = 1e-5

DC = D // P       # 8 feature chunks of 128
LC = L // P       # 16 key chunks of 128
QS = NQ // P      # 4 query chunks of 128
KC = L // NQ      # 4 key-column chunks of 512

SW = 16.0                      # weight upscale for fp8 range
CEXP = (0.25, 1.0 / 64.0)      # e^{bias} per key group G
BEXP = (math.log(CEXP[0]), math.log(CEXP[1]))
AEXP = tuple(SCALE * (2.0 ** G) / (SW * SW) for G in range(QG))
MVAL = 0.25                    # masked-ones value (denominator scale)
R = (16.0 / MVAL) * SW         # 1024: global scale on pre-LN activations
EPS_S = R * R * LN_EPS

F8 = mybir.dt.float8e4
BF16 = mybir.dt.bfloat16
F32 = mybir.dt.float32
NPF8 = ml_dtypes.float8_e4m3
NPBF16 = ml_dtypes.bfloat16
DR = mybir.MatmulPerfMode.DoubleRow


def _patch_tile_drain():
    """The public neuronxcc walrus build rejects instructions with more than
    one semaphore wait ("Too many sync wait commands"). Tile's kernel-tail
    drain waits on every used proc's final tick, so split it into a chain of
    single-wait drains on the sync engine."""

    def _drain_and_barrier_split(self, tick_clock, wait_clock):
        nc = self.nc
        drain_inst = nc.sync.drain()
        wait_clock.add_sem_waits(
            drain_inst.ins, ScopedClock({None: tick_clock.global_clock})
        )
        si = drain_inst.ins.sync_info
        if si is not None and len(si.on_wait) > 1:
            waits = list(si.on_wait)
            updates = list(si.on_update)
            drain_inst.ins.sync_info = mybir.SyncInfo(
                on_wait=[waits[0]], on_update=updates
            )
            for w in waits[1:]:
                d2 = nc.sync.drain()
                d2.ins.sync_info = mybir.SyncInfo(on_wait=[w], on_update=[])

        nc.all_engine_barrier()
        assert self.sems is not None
        popped = nc._tile_sem_poison_stack.pop()
        assert popped is self._sem_poison
        nc.clear_and_free_semaphores(list(self.sems.allocated().values()))
        nc.all_engine_barrier()

    tile.TileContext._drain_and_barrier = _drain_and_barrier_split


_patch_tile_drain()


def _split_multi_waits(nc):
    """The public neuronxcc walrus build supports only ONE semaphore wait per
    instruction. Tile's sem-assigner can put several waits on one
    instruction; hoist the extras onto same-engine NoOps inserted right
    before it (engines execute in block order, so waiting sequentially is
    equivalent)."""
    k = 0
    for f in nc.m.functions:
        for bb in f.blocks:
            new = []
            changed = False
            for inst in bb.instructions:
                si = inst.sync_info
                if si is not None and len(si.on_wait) > 1:
                    waits = list(si.on_wait)
                    for w in waits[:-1]:
                        nop = mybir.InstNoOp(
                            name=f"wsplit-{k}",
                            engine=inst.engine,
                            ins=[],
                            outs=[],
                            sync_info=mybir.SyncInfo(on_wait=[w], on_update=[]),
                        )
                        new.append(nop)
                        k += 1
                    inst.sync_info = mybir.SyncInfo(
                        on_wait=[waits[-1]], on_update=list(si.on_update)
                    )
                    changed = True
                new.append(inst)
            if changed:
                bb.instructions = new


def build_nc() -> bass.Bass:
    nc = bass.Bass("TRN2", num_devices=8)

    xqT = nc.dram_tensor("xqT", [DC, P, NQ], F8, kind="ExternalInput")
    xkT = nc.dram_tensor("xkT", [DC, P, NQ], F8, kind="ExternalInput")
    xfT = nc.dram_tensor("xfT", [DC, P, L], F8, kind="ExternalInput")
    wkT = nc.dram_tensor("wkT", [DC, P, D], F8, kind="ExternalInput")
    wvT = nc.dram_tensor("wvT", [DC, P, D], F8, kind="ExternalInput")
    wqT = nc.dram_tensor("wqT", [DC, P, D], F8, kind="ExternalInput")
    woT = nc.dram_tensor("woT", [DC, P, D], F8, kind="ExternalInput")
    xres = nc.dram_tensor("xres", [QS, P, D], BF16, kind="ExternalInput")
    maskones = nc.dram_tensor("maskones", [P, LC, P], F8, kind="ExternalInput")
    out = nc.dram_tensor("out", [QS, P, D], F32, kind="ExternalOutput")
    # V AllGather bounce buffers (per-batch quad).
    vcc_in = nc.dram_tensor("vcc_in", [QS * P, D], F8, kind="Internal")
    vcc_out = nc.dram_tensor("vcc_out", [4 * QS * P, D], F8, kind="Internal")
    RG = [[0, 1, 2, 3], [4, 5, 6, 7]]

    with (
        tile.TileContext(nc) as tc,
        tc.tile_pool(name="persist", bufs=1) as persist,
    ):
        # ---- persistent SBUF tiles
        wkT_sb = persist.tile([P, DC, D], F8)
        wvT_sb = persist.tile([P, DC, D], F8)
        wqT_sb = persist.tile([P, DC, D], F8)
        woT_sb = persist.tile([P, DC, D], F8)
        xqT_sb = persist.tile([P, DC, NQ], F8)
        xkT_sb = persist.tile([P, DC, NQ], F8)     # own keys, masked
        xfT_sb = persist.tile([P, DC, L], F8)      # all keys, masked
        # K^T full, one tensor per 512-key column chunk so the per-chunk
        # copybacks never cross-engine-serialize on a shared tensor
        kT_sb = [persist.tile([P, DC, NQ], F8, name=f"kT{kc}")
                 for kc in range(KC)]
        v_sb = persist.tile([P, LC, D], F8)        # V full [key, feat]
        qT_sb = persist.tile([P, DC, NQ], F8)      # Q^T [feat, q]
        vsh_sb = persist.tile([P, QS, D], F8)      # own V shard
        pt_sb = persist.tile([P, QG, QG, LC, NQ], F8)  # exp(scores) [g][G]
        t_sb = persist.tile([P, QG, QS, NQ], F32)  # normalized PV out per G
        ucb_sb = persist.tile([P, DC, NQ], F8)     # attn out^T, both g halves
        xres_sb = persist.tile([P, QS, D], BF16)
        mones_sb = persist.tile([P, LC, P], F8)
        rec_sb = persist.tile([1, 2 * QG, NQ], BF16)
        onescol = persist.tile([1, P], BF16)
        b0_sb = persist.tile([P, 1], F32)
        b1_sb = persist.tile([P, 1], F32)
        eps_sb = persist.tile([P, 1], F32)
        zero_sb = persist.tile([P, 1], F32)

        # ---- input DMAs. The DMA device is effectively serial, so order by
        # criticality: V-projection inputs first (its AllGather must launch
        # early), then Q, then the big full-batch K inputs, then phase-D data.
        nc.sync.dma_start(xkT_sb[:], xkT[:].rearrange("c p n -> p c n"))
        for h in range(4):
            nc.sync.dma_start(
                wvT_sb[:, 2 * h:2 * h + 2, :],
                wvT[2 * h:2 * h + 2].rearrange("c p n -> p c n"),
            )
        for h in range(4):
            nc.sync.dma_start(
                wkT_sb[:, 2 * h:2 * h + 2, :],
                wkT[2 * h:2 * h + 2].rearrange("c p n -> p c n"),
            )
        nc.sync.dma_start(xqT_sb[:], xqT[:].rearrange("c p n -> p c n"))
        nc.sync.dma_start(wqT_sb[:], wqT[:].rearrange("c p n -> p c n"))
        nc.sync.dma_start(mones_sb[:], maskones[:])
        for h in range(8):
            nc.sync.dma_start(
                xfT_sb[:, h, :], xfT[h],
            )
        nc.sync.dma_start(woT_sb[:], woT[:].rearrange("c p n -> p c n"))
        nc.sync.dma_start(xres_sb[:], xres[:].rearrange("s p n -> p s n"))
        nc.vector.memset(onescol[:], 1.0)
        nc.vector.memset(b0_sb[:], BEXP[0])
        nc.vector.memset(b1_sb[:], BEXP[1])
        nc.vector.memset(eps_sb[:], EPS_S)
        nc.vector.memset(zero_sb[:], 0.0)
        bias_sb = (b0_sb, b1_sb)

        pcb_i = 0

        def pcb(dst, ps, eng=None):
            # psum->sbuf copyback; alternate DVE/ACT so neither gates PSUM
            nonlocal pcb_i
            if eng is None:
                eng = "v" if pcb_i % 2 == 0 else "s"
                pcb_i += 1
            if eng == "v":
                nc.vector.tensor_copy(out=dst, in_=ps[:])
            else:
                nc.scalar.copy(out=dst, in_=ps[:])

        # ================= phase A: projections + V AllGather
        with tc.tile_pool(name="pa", bufs=1, space="PSUM") as pa:
            warm_ps = pa.tile([P, P], F32, tag="warm", name="warm_ps")

            def warm(n):
                # tiny self-contained matmuls keep the PE p-state pinned at
                # full clock through DMA-paced stretches
                for _ in range(n):
                    nc.tensor.matmul(
                        warm_ps[:], lhsT=onescol[:], rhs=onescol[:],
                        start=True, stop=True,
                    )

            def proj_pass(n_out, mk_mm, mk_dst, eng=None):
                """c-pair-outer projection over n_out 512-wide outputs using
                7 psum banks (+1 short tail pass), warm trickle between
                groups."""
                tiles = [pa.tile([P, NQ], F32, tag=f"a{j % 7}",
                                 name=f"pp{j % 7}")
                         for j in range(min(n_out, 7))]
                for cp in range(4):
                    for j in range(min(n_out, 7)):
                        mk_mm(tiles[j], j, cp)
                    warm(2)
                for j in range(min(n_out, 7)):
                    pcb(mk_dst(j), tiles[j], eng=eng)
                if n_out == 8:
                    t7 = pa.tile([P, NQ], F32, tag="a0", name="pp7")
                    for cp in range(4):
                        mk_mm(t7, 7, cp)
                        warm(1)
                    pcb(mk_dst(7), t7, eng=eng)

            warm(28)

            # V shard [own 512 keys, feat], both G halves
            def v_mm(t, j, cp):
                G, ls = j // 4, j % 4
                nc.tensor.matmul(
                    t[:],
                    lhsT=xkT_sb[:, 2 * cp:2 * cp + 2, ls * P:(ls + 1) * P],
                    rhs=wvT_sb[:, 2 * cp:2 * cp + 2, G * GD:(G + 1) * GD],
                    start=(cp == 0), stop=(cp == 3), perf_mode=DR,
                )

            proj_pass(8, v_mm,
                      lambda j: vsh_sb[:, j % 4, (j // 4) * GD:
                                       (j // 4 + 1) * GD], eng="v")
            vcc_in_t = bass.AP(
                tensor=vcc_in, offset=0,
                ap=[[D, P], [P * D, QS], [1, D]],
            )
            nc.gpsimd.dma_start(vcc_in_t, vsh_sb[:])
            nc.gpsimd.collective_compute(
                "AllGather", mybir.AluOpType.bypass, replica_groups=RG,
                ins=[vcc_in[:]], outs=[vcc_out[:]],
            )
            for r in range(4):
                src = bass.AP(
                    tensor=vcc_out, offset=r * QS * P * D,
                    ap=[[D, P], [P * D, QS], [1, D]],
                )
                nc.sync.dma_start(v_sb[:, r * QS:(r + 1) * QS, :], src)

            # Q^T [feat, q]
            def q_mm(t, j, cp):
                nc.tensor.matmul(
                    t[:],
                    lhsT=wqT_sb[:, 2 * cp:2 * cp + 2, j * P:(j + 1) * P],
                    rhs=xqT_sb[:, 2 * cp:2 * cp + 2, :],
                    start=(cp == 0), stop=(cp == 3), perf_mode=DR,
                )

            proj_pass(8, q_mm, lambda j: qT_sb[:, j, :], eng="s")

            # K^T full, key-column chunk kc=0 (kc 1-3 interleave into phase B)
            def k_mm(kc):
                def mm(t, j, cp):
                    nc.tensor.matmul(
                        t[:],
                        lhsT=wkT_sb[:, 2 * cp:2 * cp + 2, j * P:(j + 1) * P],
                        rhs=xfT_sb[:, 2 * cp:2 * cp + 2,
                                   kc * NQ:(kc + 1) * NQ],
                        start=(cp == 0), stop=(cp == 3), perf_mode=DR,
                    )
                return mm

            def k_dst(kc):
                return lambda j: kT_sb[kc][:, j, :]

            warm(24)
            proj_pass(8, k_mm(0), k_dst(0))
            warm(8)

        # ===== phase B: scores/softmax/PV, software-piped with the
        # remaining K projection chunks. The exp stream on ACT is the
        # spine; previous groups' denominator/PV matmuls and K kc-chunks
        # 1-3 fill the PE gaps between exp-paced fills.
        if True:
            with (
                tc.tile_pool(name="ps_s", bufs=1, space="PSUM") as ps_s,
                tc.tile_pool(name="ps_mm", bufs=2, space="PSUM") as ps_mm,
                tc.tile_pool(name="ps_d", bufs=1, space="PSUM") as ps_d,
                tc.tile_pool(name="rbc_pool", bufs=1) as rbc_pool,
            ):
                s_tiles = {}

                def emit_fill(g, G, p):
                    # 2-key-chunk fill + [128,1024] exp; the two (g,G0)/(g,G1)
                    # streams interleave on separate psum tiles so the ACT exp
                    # stream never waits on a fill round-trip
                    if p == 0:
                        s_tiles[(g, G)] = ps_s.tile([P, 2, NQ], F32,
                                                    tag=f"s{G}", name="s_ps")
                    s_ps = s_tiles[(g, G)]
                    for ks in (2 * p, 2 * p + 1):
                        kc, ki = ks // 4, ks % 4
                        for j in range(2):
                            nc.tensor.matmul(
                                s_ps[:, ks % 2, :],
                                lhsT=kT_sb[kc][:, G * 4 + 2 * j:
                                               G * 4 + 2 * j + 2,
                                               ki * P:(ki + 1) * P],
                                rhs=qT_sb[:, g * 4 + 2 * j:
                                          g * 4 + 2 * j + 2, :],
                                start=(j == 0), stop=(j == 1), perf_mode=DR,
                            )
                    nc.scalar.activation(
                        out=pt_sb[:, g, G, 2 * p:2 * p + 2, :],
                        in_=s_ps[:],
                        func=mybir.ActivationFunctionType.Exp,
                        bias=bias_sb[G][:],
                        scale=AEXP[G],
                    )

                def emit_kc_chunks(kc, gc0, n):
                    # gc-chunks of one K key-column chunk; copybacks on DVE
                    # only (ACT is saturated with exps here)
                    for gc in range(gc0, gc0 + n):
                        t = ps_mm.tile([P, NQ], F32, tag="mm", name="k_ps")
                        for cp in range(4):
                            nc.tensor.matmul(
                                t[:],
                                lhsT=wkT_sb[:, 2 * cp:2 * cp + 2,
                                            gc * P:(gc + 1) * P],
                                rhs=xfT_sb[:, 2 * cp:2 * cp + 2,
                                           kc * NQ:(kc + 1) * NQ],
                                start=(cp == 0), stop=(cp == 3), perf_mode=DR,
                            )
                        pcb(kT_sb[kc][:, gc, :], t, eng="v")

                warm_i = [0]

                def warm_b(n):
                    # keep the PE p-state pinned through dependency waits
                    # (borrows the rbc psum slot, free between PV rounds)
                    wt = ps_d.tile([P, P], F32, tag="r", name="wt")
                    warm_i[0] += 1
                    for _ in range(n):
                        nc.tensor.matmul(
                            wt[:], lhsT=onescol[:], rhs=onescol[:],
                            start=True, stop=True,
                        )

                def emit_denom(g, G, warm=0):
                    d_ps = ps_d.tile([P, NQ], F32, tag="d", name="d_ps")
                    for kp in range(8):
                        nc.tensor.matmul(
                            d_ps[:],
                            lhsT=mones_sb[:, 2 * kp:2 * kp + 2, :],
                            rhs=pt_sb[:, g, G, 2 * kp:2 * kp + 2, :],
                            start=(kp == 0), stop=(kp == 7), perf_mode=DR,
                        )
                        if warm:
                            warm_b(warm)
                    ri = g * QG + G
                    with nc.allow_low_precision(reason="bf16 softmax recip"):
                        nc.vector.reciprocal(out=rec_sb[:, ri, :],
                                             in_=d_ps[0:1, :])

                def emit_rbc(g, G):
                    ri = g * QG + G
                    r_ps = ps_d.tile([P, NQ], F32, tag="r", name="r_ps")
                    nc.tensor.matmul(
                        r_ps[:], lhsT=onescol[:], rhs=rec_sb[:, ri, :],
                        start=True, stop=True,
                    )
                    # hardware tensor ops read at most one PSUM operand, so
                    # stage the broadcast reciprocal in SBUF for the PV muls
                    rb = rbc_pool.tile([P, NQ], F32, tag="rbc", bufs=2,
                                       name="rb")
                    nc.vector.tensor_copy(out=rb[:], in_=r_ps[:])
                    return rb

                def emit_pv_ds(g, G, ds, r_ps, warm=0):
                    u_ps = ps_mm.tile([P, NQ], F32, tag="mm", name="u_ps")
                    for kp in range(8):
                        nc.tensor.matmul(
                            u_ps[:],
                            lhsT=v_sb[:, 2 * kp:2 * kp + 2,
                                      G * GD + ds * P:G * GD + (ds + 1) * P],
                            rhs=pt_sb[:, g, G, 2 * kp:2 * kp + 2, :],
                            start=(kp == 0), stop=(kp == 7), perf_mode=DR,
                        )
                        if warm:
                            warm_b(warm)
                    nc.vector.tensor_tensor(
                        out=t_sb[:, G, ds, :], in0=u_ps[:], in1=r_ps[:],
                        op=mybir.AluOpType.mult,
                    )

                def emit_gadd(g):
                    # t(G0) + t(G1) -> fp8 attn-out chunks, split across DVE
                    # and Pool so the sum is off the critical path quickly
                    for ds in range(QS):
                        nc.vector.tensor_add(
                            out=ucb_sb[:, g * 4 + ds, :],
                            in0=t_sb[:, 0, ds, :], in1=t_sb[:, 1, ds, :],
                        )

                z_sb = persist.tile([P, QS, D], F32, name="z_sb")
                mv = persist.tile([P, QS, 2], F32, name="mv")
                rstd = persist.tile([P, QS], F32, name="rstd")
                nmr = persist.tile([P, QS], F32, name="nmr")
                o_dve = persist.tile([P, 2, D], F32, name="o_dve")
                o_act = persist.tile([P, 2, D], F32, name="o_act")
                stats_sb = persist.tile([P, 2, 6], F32, name="stats_sb")

                def emit_ln_final(qs):
                    # rstd = exp(-0.5*ln(var+eps)) -- ln/exp share the ACT
                    # table with the softmax exps, so no mid-kernel reload
                    nc.scalar.activation(
                        out=rstd[:, qs:qs + 1], in_=mv[:, qs, 1:2],
                        func=mybir.ActivationFunctionType.Ln,
                        bias=eps_sb[:], scale=1.0,
                    )
                    nc.scalar.activation(
                        out=rstd[:, qs:qs + 1], in_=rstd[:, qs:qs + 1],
                        func=mybir.ActivationFunctionType.Exp,
                        bias=zero_sb[:], scale=-0.5,
                    )
                    o_half = (o_dve if qs % 2 == 0 else o_act)[:, qs // 2, :]
                    if qs % 2 == 0:
                        nc.vector.tensor_scalar(
                            out=o_half, in0=z_sb[:, qs, :],
                            scalar1=mv[:, qs, 0:1], scalar2=rstd[:, qs:qs + 1],
                            op0=mybir.AluOpType.subtract,
                            op1=mybir.AluOpType.mult,
                        )
                    else:
                        nc.vector.tensor_scalar(
                            out=nmr[:, qs:qs + 1], in0=mv[:, qs, 0:1],
                            scalar1=rstd[:, qs:qs + 1], scalar2=-1.0,
                            op0=mybir.AluOpType.mult,
                            op1=mybir.AluOpType.mult,
                        )
                        nc.scalar.activation(
                            out=o_half, in_=z_sb[:, qs, :],
                            func=mybir.ActivationFunctionType.Identity,
                            bias=nmr[:, qs:qs + 1], scale=rstd[:, qs:qs + 1],
                        )
                    nc.sync.dma_start(out[qs], o_half)

                def emit_oproj(half, with_bn):
                    # O-proj round for one g-half of the attn features,
                    # accumulated into z via DVE adds (residual rides the
                    # first round); bn_stats trail the second round
                    for qs in range(QS):
                        for js in range(2):
                            y_ps = ps_mm.tile([P, NQ], F32, tag="mm",
                                              name="y_ps")
                            for j in (2 * half, 2 * half + 1):
                                nc.tensor.matmul(
                                    y_ps[:],
                                    lhsT=ucb_sb[:, 2 * j:2 * j + 2,
                                                qs * P:(qs + 1) * P],
                                    rhs=woT_sb[:, 2 * j:2 * j + 2,
                                               js * GD:(js + 1) * GD],
                                    start=(j == 2 * half),
                                    stop=(j == 2 * half + 1),
                                    perf_mode=DR,
                                )
                            other = (xres_sb[:, qs, js * GD:(js + 1) * GD]
                                     if half == 0 else
                                     z_sb[:, qs, js * GD:(js + 1) * GD])
                            nc.vector.tensor_add(
                                out=z_sb[:, qs, js * GD:(js + 1) * GD],
                                in0=y_ps[:], in1=other,
                            )
                        if with_bn:
                            for h in range(2):
                                nc.vector.bn_stats(
                                    out=stats_sb[:, h, :],
                                    in_=z_sb[:, qs, h * GD:(h + 1) * GD],
                                )
                            nc.vector.bn_aggr(out=mv[:, qs, :],
                                              in_=stats_sb[:])
                            emit_ln_final(qs)

                # ---- schedule: per g, interleave the G0/G1 fill+exp streams;
                # K kc-chunks 1-3, denominators and PVs of earlier groups fill
                # the PE gaps. PV lags so the V AllGather stays off the
                # critical path.
                # kc chunk c must be fully in SBUF before the p=2c
                # fills read it: emit the G1-half (gc4-7) two slots ahead
                # and the G0-half one slot ahead of first use
                kc_sched = {0: (1, 4), 1: (1, 0), 2: (2, 4), 3: (2, 0),
                            4: (3, 4), 5: (3, 0)}
                for p in range(8):        # g = 0
                    emit_fill(0, 0, p)
                    emit_fill(0, 1, p)
                    if p in kc_sched:
                        kc, gc0 = kc_sched[p]
                        emit_kc_chunks(kc, gc0, 4)
                rp = None
                for p in range(8):        # g = 1
                    emit_fill(1, 0, p)
                    emit_fill(1, 1, p)
                    if p == 0:
                        emit_denom(0, 0)
                    elif p == 2:
                        rp = emit_rbc(0, 0)
                        emit_pv_ds(0, 0, 0, rp)
                    elif p == 3:
                        emit_pv_ds(0, 0, 1, rp)
                    elif p == 4:
                        emit_pv_ds(0, 0, 2, rp)
                        emit_pv_ds(0, 0, 3, rp)
                    elif p == 5:
                        emit_denom(0, 1)
                        rp = emit_rbc(0, 1)
                        emit_pv_ds(0, 1, 0, rp)
                    elif p == 6:
                        emit_pv_ds(0, 1, 1, rp)
                        emit_pv_ds(0, 1, 2, rp)
                    elif p == 7:
                        emit_pv_ds(0, 1, 3, rp)
                # tail: g0 attn-out + first O-proj round overlap the last
                # exps; (1,1) denominator/PV matmuls trail the exp stream
                # incrementally
                emit_gadd(0)
                emit_oproj(0, with_bn=False)
                emit_denom(1, 0)
                rp = emit_rbc(1, 0)
                for ds in range(QS):
                    emit_pv_ds(1, 0, ds, rp)
                emit_denom(1, 1)
                rp = emit_rbc(1, 1)
                for ds in range(QS):
                    emit_pv_ds(1, 1, ds, rp)
                emit_gadd(1)
                emit_oproj(1, with_bn=True)

    _split_multi_waits(nc)
    return nc


def make_in_maps(x, mask, Wq, Wkv, Wo, ln_g, ln_b):
    """Host-side prep: per-core transposed/fp8/pre-permuted input arrays."""
    x = np.asarray(x, np.float32)
    mask = np.asarray(mask)
    Wq = np.asarray(Wq, np.float32)
    Wkv = np.asarray(Wkv, np.float32)
    Wo = np.asarray(Wo, np.float32)

    def f8(a):
        return np.clip(a, -240.0, 240.0).astype(NPF8)

    # Wkv rows: K features (G-major: G, h, d), V features (G-major).
    A = Wkv.reshape(HEADS, 2, P, D)
    kw = A[:, 0].reshape(D, D)
    vw = A[:, 1].reshape(D, D)
    wkT = np.ascontiguousarray(f8(SW * kw.T).reshape(DC, P, D))
    wvT = np.ascontiguousarray(f8(SW * vw.T).reshape(DC, P, D))
    wqT = f8(SW * Wq.T).reshape(DC, P, D).copy()
    woT = f8(SW * Wo.T).reshape(DC, P, D).copy()

    in_maps = []
    for core in range(8):
        b, qc = core // 4, core % 4
        q0 = qc * NQ
        xb = x[b]                                     # [L, D]
        keep = (~mask[b]).astype(np.float32)          # 1 = unmasked
        xmask = xb * keep[:, None]
        xq = xb[q0:q0 + NQ]
        xqT = f8(xq.T).reshape(DC, P, NQ).copy()
        xkT = np.ascontiguousarray(
            f8(xmask[q0:q0 + NQ].T).reshape(DC, P, NQ))
        xfT = np.ascontiguousarray(f8(xmask.T).reshape(DC, P, L))
        xres = (R * xq).astype(NPBF16).reshape(QS, P, D).copy()
        maskones = np.ascontiguousarray(np.repeat(
            (keep.reshape(LC, P).T * MVAL).astype(NPF8)[:, :, None], P, axis=2
        ))
        in_maps.append({
            "xqT": xqT, "xkT": xkT, "xfT": xfT,
            "wkT": wkT, "wvT": wvT, "wqT": wqT, "woT": woT,
            "xres": xres, "maskones": maskones,
        })
    return in_maps


_NC_CACHE = {}


def get_nc() -> bass.Bass:
    if "nc" not in _NC_CACHE:
        _NC_CACHE["nc"] = build_nc()
    return _NC_CACHE["nc"]


def kernel(**inputs) -> np.ndarray:
    from concourse.bass_utils import run_bass_kernel_spmd

    in_maps = make_in_maps(
        inputs["x"], inputs["mask"], inputs["Wq"], inputs["Wkv"],
        inputs["Wo"], inputs["ln_g"], inputs["ln_b"],
    )
    nc = get_nc()
    B = 2
    full = np.empty((B, L, D), np.float32)
    for attempt in range(3):
        res = run_bass_kernel_spmd(nc, in_maps, core_ids=list(range(8)))
        for core in range(8):
            b, qc = core // 4, core % 4
            full[b, qc * NQ:(qc + 1) * NQ] = \
                res.results[core]["out"].reshape(NQ, D)
        # rare cross-core sync flake under the emulated NRT can surface as
        # NaN; the dispatch is cheap relative to a wrong answer, so retry
        if np.isfinite(full).all():
            break
    ln_g = np.asarray(inputs["ln_g"], np.float32)
    ln_b = np.asarray(inputs["ln_b"], np.float32)
    if not (ln_g == 1.0).all() or not (ln_b == 0.0).all():
        full = full * ln_g + ln_b
    return full
